# revision 9
# baseline (speedup 1.0000x reference)
"""YOLOv1 loss kernel for 8 Trainium2 NeuronCores.

Strategy (data-parallel, per spec sharding hint):
  - Shard the batch dim (32768) across 8 cores -> 4096 samples/core.
  - Each core computes a per-partition partial sum of the loss terms; the
    host does the final (tiny) reduction across 8*128*2 floats in float64.

Math notes (validated against the jax reference):
  - The grid offsets (m, n) cancel inside the IoU (all box corners share the
    same +m/G, +n/G shift), so no iota/grid constants are needed.
  - IoU is computed in 7x-scaled coordinates: corners c +/- 3.5w, areas
    scaled by 49: iou = i7/u7 with i7 = relu(iw)*relu(ih), u7 = 49*(ap+ag)-i7.
  - where(inter>0, inter/union, 0) is automatic: i7 == 0 -> iou == 0.
  - total = sum(obj*(sel + cls - 0.5*nq)) + 0.5*sum(nq),  nq = p4^2 + p9^2.

Engine split (v2, tuned from the NTFF profile of v1):
  - DVE runs tensor_tensor / tensor_scalar ops in bf16 (2x packed mode);
    scalar_tensor_tensor (1x-only) ops eliminated except the fp32 union.
  - ACT runs the LUT ops (sqrt, square, relu) plus the scale-by-constant
    copies (3.5*wh), using its idle capacity.
  - The reciprocal stays the fp32 DVE approx (ACT Reciprocal is blocked).

Layout: partition = sample block (128), free = [samples(8), channels, cells].
"""

import numpy as np

import concourse.bacc as bacc
import concourse.tile as tile
from concourse import mybir
from concourse.bass_utils import run_bass_kernel_spmd

# Problem constants (hardcoded per contract; kernel.py must be self-contained).
B = 32768
N_CORES = 8
BC = B // N_CORES            # 4096 samples per core
P = 128                      # SBUF partitions
S = 8                        # samples per partition per block
NBLK = BC // (P * S)         # 4 blocks per core
K = 49                       # grid cells (7*7)

F32 = mybir.dt.float32
BF16 = mybir.dt.bfloat16

AL = mybir.AluOpType
AF = mybir.ActivationFunctionType


def _build(nblk=NBLK):
    nc = bacc.Bacc("TRN2", target_bir_lowering=False, debug=False,
                   num_devices=N_CORES)
    bc = nblk * P * S
    pred = nc.dram_tensor("pred", [bc, 30, K], F32, kind="ExternalInput")
    labels = nc.dram_tensor("labels", [bc, 30, K], F32, kind="ExternalInput")
    out = nc.dram_tensor("acc", [P, 2], F32, kind="ExternalOutput")

    pred_r = pred.ap().rearrange("(t p s) c k -> t p s c k", p=P, s=S)
    lab_r = labels.ap().rearrange("(t p s) c k -> t p s c k", p=P, s=S)

    with tile.TileContext(nc) as tc:
        with (
            tc.tile_pool(name="io", bufs=2) as io,
            tc.tile_pool(name="quadp", bufs=10) as wk,
            tc.tile_pool(name="bip", bufs=10) as bip,
            tc.tile_pool(name="unitp", bufs=12) as unitp,
            tc.tile_pool(name="ufp", bufs=3) as ufp,
            tc.tile_pool(name="treep", bufs=2) as treep,
            tc.tile_pool(name="accp", bufs=1) as accp,
        ):
            ACC = accp.tile([P, S, K], F32, tag="ACC")
            ACCN = accp.tile([P, S, K], F32, tag="ACCN")
            nc.vector.memset(ACC, 0.0)
            nc.vector.memset(ACCN, 0.0)

            for t in range(nblk):
                _block(nc, io, wk, bip, unitp, ufp, treep, ACC, ACCN,
                       pred_r[t], lab_r[t])

            # ---- final per-core reduce: [P,S,K] -> [P,1] each ----
            red = accp.tile([P, 2], F32, tag="red")
            nc.vector.tensor_reduce(out=red[:, 0:1], in_=ACC[:],
                                    axis=mybir.AxisListType.XY, op=AL.add)
            nc.vector.tensor_reduce(out=red[:, 1:2], in_=ACCN[:],
                                    axis=mybir.AxisListType.XY, op=AL.add)
            nc.sync.dma_start(out=out.ap(), in_=red)

    nc.finalize()
    return nc


def _block(nc, io, wk, bip, unitp, ufp, treep, ACC, ACCN, pred_t, lab_t):
    """Process one block of P*S samples. pred_t/lab_t: [P, S, 30, K] DRAM."""
    import concourse.bass as bass

    def box_view(base):
        # [P, S, 2, 6, K]: box b reads channels {5b..5b+5} (one-channel overlap
        # pads each box block to 6 channels so quad views keep even strides).
        ap = [list(x) for x in base.ap]
        return bass.AP(tensor=base.tensor, offset=base.offset,
                       ap=[ap[0], ap[1], [5 * K, 2], [K, 6], [1, K]])

    # ---- input DMAs (SWDGE cast fp32 -> bf16) ----
    # pbox/lbox triple-buffered: SWDGE processes DMAs in order on one queue,
    # so a WAR wait on the box buffers head-of-line-blocks every later DMA.
    pbox = io.tile([P, S, 2, 6, K], BF16, tag="pbox", bufs=3)
    lbox = io.tile([P, S, 2, 6, K], BF16, tag="lbox", bufs=3)
    pcls = io.tile([P, S, 20, K], BF16, tag="pcls")
    lcls = io.tile([P, S, 20, K], BF16, tag="lcls")
    nc.gpsimd.dma_start(out=pbox, in_=box_view(pred_t))
    nc.gpsimd.dma_start(out=lbox, in_=box_view(lab_t))
    nc.gpsimd.dma_start(out=pcls, in_=pred_t[:, :, 10:30, :])
    nc.gpsimd.dma_start(out=lcls, in_=lab_t[:, :, 10:30, :])

    pb, lb = pbox[:], lbox[:]
    p_c = pb[:, :, :, 0:2, :]       # pred centers x,y  (quad [P,S,2,2,K])
    p_wh = pb[:, :, :, 2:4, :]      # pred w,h          (quad)
    p49 = pb[:, :, :, 4, :]         # conf p4,p9        (bi [P,S,2,K])
    l_c = lb[:, :, 0, 0:2, :]       # gt centers        (pair [P,S,2,K])
    l_wh = lb[:, :, 0, 2:4, :]      # gt w,h            (pair)
    l_c56 = lb[:, :, :, 0:2, :]     # labels ch{0,1,5,6} (quad)
    l_wh78 = lb[:, :, :, 2:4, :]    # labels ch{2,3,7,8} (quad)
    l4 = lb[:, :, 0, 4, :]          # obj mask          (unit [P,S,K])

    QUAD, BI, UNIT = (P, S, 2, 2, K), (P, S, 2, K), (P, S, K)

    def qt(tag="quad"):
        return wk.tile([P, S, 2, 2, K], BF16, tag="quad", name=f"q_{tag}")

    def bt(tag="bi"):
        return bip.tile([P, S, 2, K], BF16, tag="bi", name=f"b_{tag}")

    def ut(tag="unit"):
        return unitp.tile([P, S, K], BF16, tag="unit", name=f"u_{tag}")

    # ---- phase 1: early consumers of the io tiles (frees them for the
    # next blocks' DMAs; SWDGE is order-sensitive) ----
    w35p = qt("w35p")
    nc.scalar.activation(out=w35p, in_=p_wh, func=AF.Copy, scale=3.5)
    w35g = bt("w35g")
    nc.scalar.activation(out=w35g, in_=l_wh, func=AF.Copy, scale=3.5)
    mask = ut("mask")
    nc.vector.tensor_single_scalar(out=mask, in_=l4, scalar=1.0, op=AL.is_equal)
    mg = ut()
    nc.vector.tensor_mul(out=mg, in0=l_wh[:, :, 0, :], in1=l_wh[:, :, 1, :])
    m = bt()
    nc.vector.tensor_mul(out=m, in0=p_wh[:, :, :, 0, :], in1=p_wh[:, :, :, 1, :])
    p49c = bt("p49c")
    nc.vector.tensor_copy(out=p49c, in_=p49)
    sq49 = bt()
    nc.scalar.square(out=sq49, in_=p49c)

    # ---- classification sub early (releases pcls/lcls), square on ACT
    # overlaps with the DVE corner chain below ----
    nc.vector.tensor_sub(out=pcls, in0=pcls, in1=lcls)
    nc.scalar.square(out=pcls, in_=pcls)

    # ---- IoU corners ----
    lo, hi = qt(), qt()
    nc.vector.tensor_sub(out=lo, in0=p_c, in1=w35p)
    nc.vector.tensor_add(out=hi, in0=p_c, in1=w35p)
    glo, ghi = bt(), bt()
    nc.vector.tensor_sub(out=glo, in0=l_c, in1=w35g)
    nc.vector.tensor_add(out=ghi, in0=l_c, in1=w35g)
    glo_b = glo[:].unsqueeze(2).to_broadcast(QUAD)
    ghi_b = ghi[:].unsqueeze(2).to_broadcast(QUAD)
    mins, maxs = qt(), qt()
    nc.vector.tensor_tensor(out=mins, in0=hi, in1=ghi_b, op=AL.min)
    nc.vector.tensor_tensor(out=maxs, in0=lo, in1=glo_b, op=AL.max)
    dd = qt()
    nc.vector.tensor_sub(out=dd, in0=mins, in1=maxs)
    dch = qt("dch")
    nc.scalar.activation(out=dch, in_=dd, func=AF.Relu)

    # coord diffs (last readers of pbox/lbox)
    d = qt("d")
    nc.vector.tensor_sub(out=d, in0=p_c, in1=l_c56)
    sp, sl = qt("sp"), qt("sl")
    nc.scalar.sqrt(out=sp, in_=p_wh)
    nc.scalar.sqrt(out=sl, in_=l_wh78)

    # cls tree head (csq ready by now)
    ta = treep.tile([P, S, 10, K], BF16, tag="ta")
    nc.vector.tensor_add(out=ta, in0=pcls[:, :, 0:10, :], in1=pcls[:, :, 10:20, :])
    tb = treep.tile([P, S, 4, K], BF16, tag="tb")
    nc.vector.tensor_add(out=tb, in0=ta[:, :, 0:4, :], in1=ta[:, :, 4:8, :])

    # ---- IoU middle ----
    i4 = bt("i4")
    nc.vector.tensor_mul(out=i4, in0=dch[:, :, :, 0, :], in1=dch[:, :, :, 1, :])
    msum = bt()
    nc.vector.tensor_add(out=msum, in0=m,
                         in1=mg[:].unsqueeze(2).to_broadcast(BI))
    # union in fp32 (ACT Reciprocal is blocked in bass; DVE approx needs fp32)
    u = ufp.tile([P, S, 2, K], F32, tag="uf", name="b_u")
    nc.vector.scalar_tensor_tensor(out=u, in0=msum, scalar=49.0, in1=i4,
                                   op0=AL.mult, op1=AL.subtract)
    r = ufp.tile([P, S, 2, K], F32, tag="uf", name="b_r")
    nc.vector.reciprocal_approx_fast(
        out=r[:].rearrange("p s b k -> p (s b k)"),
        in_=u[:].rearrange("p s b k -> p (s b k)"))
    iou = bt("iou")
    nc.vector.tensor_mul(out=iou, in0=i4, in1=r)

    # ---- coordinate loss tail ----
    dsq = qt()
    nc.vector.tensor_sub(out=dsq, in0=sp, in1=sl)
    sqd, sqds = qt(), qt()
    nc.scalar.square(out=sqd, in_=d)
    nc.scalar.square(out=sqds, in_=dsq)
    s12 = qt()
    nc.vector.tensor_add(out=s12, in0=sqd, in1=sqds)
    tab = bt("tab")
    nc.vector.tensor_add(out=tab, in0=s12[:, :, :, 0, :], in1=s12[:, :, :, 1, :])

    # ---- confidence + selection ----
    e49 = bt()
    nc.vector.tensor_sub(out=e49, in0=p49c, in1=iou)
    esq = bt("esq")
    nc.scalar.square(out=esq, in_=e49)
    x5 = bt()
    nc.vector.tensor_scalar_mul(out=x5, in0=tab, scalar1=5.0)
    x = bt("x")
    nc.vector.tensor_add(out=x, in0=x5, in1=esq)
    he = bt("he")
    nc.vector.tensor_scalar_mul(out=he, in0=esq, scalar1=0.5)
    lb1, lb2 = ut(), ut()
    nc.vector.tensor_add(out=lb1, in0=x[:, :, 0, :], in1=he[:, :, 1, :])
    nc.vector.tensor_add(out=lb2, in0=x[:, :, 1, :], in1=he[:, :, 0, :])
    resp = ut()
    nc.vector.tensor_tensor(out=resp, in0=iou[:, :, 0, :], in1=iou[:, :, 1, :],
                            op=AL.is_ge)
    dlb = ut()
    nc.vector.tensor_sub(out=dlb, in0=lb1, in1=lb2)
    sd = ut()
    nc.vector.tensor_mul(out=sd, in0=dlb, in1=resp)
    sel = ut("sel")
    nc.vector.tensor_add(out=sel, in0=lb2, in1=sd)

    nq = ut("nq")
    nc.vector.tensor_add(out=nq, in0=sq49[:, :, 0, :], in1=sq49[:, :, 1, :])

    # ---- cls tree tail ----
    tc2 = bt("tc2")
    nc.vector.tensor_add(out=tc2, in0=tb[:, :, 0:2, :], in1=tb[:, :, 2:4, :])
    td = ut()
    nc.vector.tensor_add(out=td, in0=tc2[:, :, 0, :], in1=tc2[:, :, 1, :])
    te = ut()
    nc.vector.tensor_add(out=te, in0=ta[:, :, 8, :], in1=ta[:, :, 9, :])
    clsc = ut("clsc")
    nc.vector.tensor_add(out=clsc, in0=td, in1=te)

    # ---- combine + accumulate ----
    nqh = ut()
    nc.vector.tensor_scalar_mul(out=nqh, in0=nq, scalar1=0.5)
    w1 = ut()
    nc.vector.tensor_sub(out=w1, in0=sel, in1=nqh)
    w2 = ut()
    nc.vector.tensor_add(out=w2, in0=clsc, in1=w1)
    wm = ut()
    nc.vector.tensor_mul(out=wm, in0=w2, in1=mask)
    nc.vector.tensor_add(out=ACC, in0=ACC, in1=wm)
    nc.vector.tensor_add(out=ACCN, in0=ACCN, in1=nq)


_NC_CACHE = None


def _get_nc():
    global _NC_CACHE
    if _NC_CACHE is None:
        _NC_CACHE = _build()
    return _NC_CACHE


def _make_in_maps(pred: np.ndarray, labels: np.ndarray):
    pred = np.ascontiguousarray(pred, dtype=np.float32).reshape(B, 30, K)
    labels = np.ascontiguousarray(labels, dtype=np.float32).reshape(B, 30, K)
    in_maps = []
    for i in range(N_CORES):
        sl = slice(i * BC, (i + 1) * BC)
        in_maps.append({"pred": pred[sl], "labels": labels[sl]})
    return in_maps


def _reduce_results(results) -> np.ndarray:
    total = np.float64(0.0)
    for i in range(N_CORES):
        acc = results[i]["acc"].astype(np.float64)
        total += acc[:, 0].sum() + 0.5 * acc[:, 1].sum()
    return np.asarray(np.float32(total / B))


def kernel(pred: np.ndarray, labels: np.ndarray) -> np.ndarray:
    nc = _get_nc()
    in_maps = _make_in_maps(pred, labels)
    res = run_bass_kernel_spmd(nc, in_maps, core_ids=list(range(N_CORES)),
                               trace=False)
    return _reduce_results(res.results)


# revision 12
# speedup vs baseline: 1.0252x; 1.0252x over previous
"""YOLOv1 loss kernel for 8 Trainium2 NeuronCores.

Strategy (data-parallel, per spec sharding hint):
  - Shard the batch dim (32768) across 8 cores -> 4096 samples/core.
  - Each core computes a per-partition partial sum of the loss terms; the
    host does the final (tiny) reduction across 8*128*2 floats in float64.

Math notes (validated against the jax reference):
  - The grid offsets (m, n) cancel inside the IoU (all box corners share the
    same +m/G, +n/G shift), so no iota/grid constants are needed.
  - IoU is computed in 7x-scaled coordinates: corners c +/- 3.5w, areas
    scaled by 49: iou = i7/u7 with i7 = relu(iw)*relu(ih), u7 = 49*(ap+ag)-i7.
  - where(inter>0, inter/union, 0) is automatic: i7 == 0 -> iou == 0.
  - total = sum(obj*(sel + cls - 0.5*nq)) + 0.5*sum(nq),  nq = p4^2 + p9^2.

Engine split (v2, tuned from the NTFF profile of v1):
  - DVE runs tensor_tensor / tensor_scalar ops in bf16 (2x packed mode);
    scalar_tensor_tensor (1x-only) ops eliminated except the fp32 union.
  - ACT runs the LUT ops (sqrt, square, relu) plus the scale-by-constant
    copies (3.5*wh), using its idle capacity.
  - The reciprocal stays the fp32 DVE approx (ACT Reciprocal is blocked).

Layout: partition = sample block (128), free = [samples(8), channels, cells].
"""

import numpy as np

import concourse.bacc as bacc
import concourse.tile as tile
from concourse import mybir
from concourse.bass_utils import run_bass_kernel_spmd

# Problem constants (hardcoded per contract; kernel.py must be self-contained).
B = 32768
N_CORES = 8
BC = B // N_CORES            # 4096 samples per core
P = 128                      # SBUF partitions
S = 8                        # samples per partition per block
NBLK = BC // (P * S)         # 4 blocks per core
K = 49                       # grid cells (7*7)

F32 = mybir.dt.float32
BF16 = mybir.dt.bfloat16

AL = mybir.AluOpType
AF = mybir.ActivationFunctionType


# Tapered block schedule: big blocks amortize per-op overhead; small
# trailing blocks shrink the end-of-kernel compute tail after the last DMA.
S_SCHED = (8, 8, 8, 4, 2, 2)            # samples/partition per block


def _build(sched=S_SCHED):
    nc = bacc.Bacc("TRN2", target_bir_lowering=False, debug=False,
                   num_devices=N_CORES)
    assert sum(sched) * P == BC
    pred = nc.dram_tensor("pred", [BC, 30, K], F32, kind="ExternalInput")
    labels = nc.dram_tensor("labels", [BC, 30, K], F32, kind="ExternalInput")
    out = nc.dram_tensor("acc", [P, 2], F32, kind="ExternalOutput")

    with tile.TileContext(nc) as tc:
        with (
            tc.tile_pool(name="io", bufs=2) as io,
            tc.tile_pool(name="quadp", bufs=10) as wk,
            tc.tile_pool(name="bip", bufs=10) as bip,
            tc.tile_pool(name="unitp", bufs=12) as unitp,
            tc.tile_pool(name="ufp", bufs=3) as ufp,
            tc.tile_pool(name="treep", bufs=2) as treep,
            tc.tile_pool(name="accp", bufs=1) as accp,
        ):
            ACC = accp.tile([P, S, K], F32, tag="ACC")
            ACCN = accp.tile([P, S, K], F32, tag="ACCN")
            nc.vector.memset(ACC, 0.0)
            nc.vector.memset(ACCN, 0.0)

            off = 0
            for si in sched:
                n = P * si
                pr = pred.ap()[off:off + n].rearrange(
                    "(p s) c k -> p s c k", p=P, s=si)
                lr = labels.ap()[off:off + n].rearrange(
                    "(p s) c k -> p s c k", p=P, s=si)
                _block(nc, io, wk, bip, unitp, ufp, treep, ACC, ACCN,
                       pr, lr, si)
                off += n

            # ---- final per-core reduce: [P,S,K] -> [P,1] each ----
            red = accp.tile([P, 2], F32, tag="red")
            nc.vector.tensor_reduce(out=red[:, 0:1], in_=ACC[:],
                                    axis=mybir.AxisListType.XY, op=AL.add)
            nc.vector.tensor_reduce(out=red[:, 1:2], in_=ACCN[:],
                                    axis=mybir.AxisListType.XY, op=AL.add)
            nc.sync.dma_start(out=out.ap(), in_=red)

    nc.finalize()
    return nc


def _block(nc, io, wk, bip, unitp, ufp, treep, ACC, ACCN, pred_t, lab_t, S):
    """Process one block of P*S samples. pred_t/lab_t: [P, S, 30, K] DRAM."""
    import concourse.bass as bass

    def box_view(base):
        # [P, S, 2, 6, K]: box b reads channels {5b..5b+5} (one-channel overlap
        # pads each box block to 6 channels so quad views keep even strides).
        ap = [list(x) for x in base.ap]
        return bass.AP(tensor=base.tensor, offset=base.offset,
                       ap=[ap[0], ap[1], [5 * K, 2], [K, 6], [1, K]])

    # ---- input DMAs (SWDGE cast fp32 -> bf16) ----
    # pbox/lbox triple-buffered: SWDGE processes DMAs in order on one queue,
    # so a WAR wait on the box buffers head-of-line-blocks every later DMA.
    pbox = io.tile([P, 8, 2, 6, K], BF16, tag="pbox", bufs=3, name="pbox")
    lbox = io.tile([P, 8, 2, 6, K], BF16, tag="lbox", bufs=3, name="lbox")
    pclsf = io.tile([P, 8, 20, K], BF16, tag="pcls", name="pcls")
    lclsf = io.tile([P, 8, 20, K], BF16, tag="lcls", name="lcls")
    pbox = pbox[:, 0:S]
    lbox = lbox[:, 0:S]
    pcls = pclsf[:, 0:S]
    lcls = lclsf[:, 0:S]
    nc.gpsimd.dma_start(out=pbox, in_=box_view(pred_t))
    nc.gpsimd.dma_start(out=lbox, in_=box_view(lab_t))
    nc.gpsimd.dma_start(out=pcls, in_=pred_t[:, :, 10:30, :])
    nc.gpsimd.dma_start(out=lcls, in_=lab_t[:, :, 10:30, :])

    pb, lb = pbox, lbox
    p_c = pb[:, :, :, 0:2, :]       # pred centers x,y  (quad [P,S,2,2,K])
    p_wh = pb[:, :, :, 2:4, :]      # pred w,h          (quad)
    p49 = pb[:, :, :, 4, :]         # conf p4,p9        (bi [P,S,2,K])
    l_c = lb[:, :, 0, 0:2, :]       # gt centers        (pair [P,S,2,K])
    l_wh = lb[:, :, 0, 2:4, :]      # gt w,h            (pair)
    l_c56 = lb[:, :, :, 0:2, :]     # labels ch{0,1,5,6} (quad)
    l_wh78 = lb[:, :, :, 2:4, :]    # labels ch{2,3,7,8} (quad)
    l4 = lb[:, :, 0, 4, :]          # obj mask          (unit [P,S,K])

    QUAD, BI = (P, S, 2, 2, K), (P, S, 2, K)

    def qt(tag="quad"):
        t = wk.tile([P, 8, 2, 2, K], BF16, tag="quad", name=f"q_{tag}")
        return t[:, 0:S]

    def bt(tag="bi"):
        t = bip.tile([P, 8, 2, K], BF16, tag="bi", name=f"b_{tag}")
        return t[:, 0:S]

    def ut(tag="unit"):
        t = unitp.tile([P, 8, K], BF16, tag="unit", name=f"u_{tag}")
        return t[:, 0:S]

    # ---- phase 1: early consumers of the io tiles ----
    # ACT queue order matters: dch (needed by the DVE IoU chain) must come
    # before the fat cls square, which runs last in this block's ACT stream.
    w35p = qt("w35p")
    nc.scalar.activation(out=w35p, in_=p_wh, func=AF.Copy, scale=3.5)
    w35g = bt("w35g")
    nc.scalar.activation(out=w35g, in_=l_wh, func=AF.Copy, scale=3.5)
    mask = ut("mask")
    nc.vector.tensor_single_scalar(out=mask, in_=l4, scalar=1.0, op=AL.is_equal)
    mg = ut()
    nc.vector.tensor_mul(out=mg, in0=l_wh[:, :, 0, :], in1=l_wh[:, :, 1, :])
    m = bt()
    nc.vector.tensor_mul(out=m, in0=p_wh[:, :, :, 0, :], in1=p_wh[:, :, :, 1, :])
    p49c = bt("p49c")
    nc.vector.tensor_copy(out=p49c, in_=p49)

    # ---- IoU corners ----
    lo, hi = qt(), qt()
    nc.vector.tensor_sub(out=lo, in0=p_c, in1=w35p)
    nc.vector.tensor_add(out=hi, in0=p_c, in1=w35p)
    glo, ghi = bt(), bt()
    nc.vector.tensor_sub(out=glo, in0=l_c, in1=w35g)
    nc.vector.tensor_add(out=ghi, in0=l_c, in1=w35g)
    glo_b = glo.unsqueeze(2).to_broadcast(QUAD)
    ghi_b = ghi.unsqueeze(2).to_broadcast(QUAD)
    mins, maxs = qt(), qt()
    nc.vector.tensor_tensor(out=mins, in0=hi, in1=ghi_b, op=AL.min)
    nc.vector.tensor_tensor(out=maxs, in0=lo, in1=glo_b, op=AL.max)
    dd = qt()
    nc.vector.tensor_sub(out=dd, in0=mins, in1=maxs)
    dch = qt("dch")
    nc.scalar.activation(out=dch, in_=dd, func=AF.Relu)

    # coord diff (last reader of pbox centers) + sqrt on ACT
    d = qt("d")
    nc.vector.tensor_sub(out=d, in0=p_c, in1=l_c56)
    sp, sl = qt("sp"), qt("sl")
    nc.scalar.sqrt(out=sp, in_=p_wh)
    nc.scalar.sqrt(out=sl, in_=l_wh78)

    # ---- classification: sub on DVE, square split ACT(14ch)/DVE(6ch) ----
    nc.vector.tensor_sub(out=pcls, in0=pcls, in1=lcls)
    nc.vector.tensor_mul(out=pcls[:, :, 14:20, :], in0=pcls[:, :, 14:20, :],
                         in1=pcls[:, :, 14:20, :])

    # ---- IoU middle ----
    i4 = bt("i4")
    nc.vector.tensor_mul(out=i4, in0=dch[:, :, :, 0, :], in1=dch[:, :, :, 1, :])
    msum = bt()
    nc.vector.tensor_add(out=msum, in0=m,
                         in1=mg.unsqueeze(2).to_broadcast(BI))
    # union in fp32 (ACT Reciprocal is blocked in bass; DVE approx needs fp32)
    u = ufp.tile([P, 8, 2, K], F32, tag="uf", name="b_u")[:, 0:S]
    nc.vector.scalar_tensor_tensor(out=u, in0=msum, scalar=49.0, in1=i4,
                                   op0=AL.mult, op1=AL.subtract)
    r = ufp.tile([P, 8, 2, K], F32, tag="uf", name="b_r")[:, 0:S]
    nc.vector.reciprocal_approx_fast(
        out=r.rearrange("p s b k -> p (s b k)"),
        in_=u.rearrange("p s b k -> p (s b k)"))
    iou = bt("iou")
    nc.vector.tensor_mul(out=iou, in0=i4, in1=r)

    # ACT stream (after dch/sp/sl): conf squares then the fat cls square
    sq49 = bt()
    nc.scalar.square(out=sq49, in_=p49c)
    nc.scalar.square(out=pcls[:, :, 0:14, :], in_=pcls[:, :, 0:14, :])

    # ---- coordinate loss tail ----
    dsq = qt()
    nc.vector.tensor_sub(out=dsq, in0=sp, in1=sl)
    sqd, sqds = qt(), qt()
    nc.scalar.square(out=sqd, in_=d)
    nc.scalar.square(out=sqds, in_=dsq)
    s12 = qt()
    nc.vector.tensor_add(out=s12, in0=sqd, in1=sqds)
    tab = bt("tab")
    nc.vector.tensor_add(out=tab, in0=s12[:, :, :, 0, :], in1=s12[:, :, :, 1, :])

    # ---- confidence + selection ----
    e49 = bt()
    nc.vector.tensor_sub(out=e49, in0=p49c, in1=iou)
    esq = bt("esq")
    nc.scalar.square(out=esq, in_=e49)
    x5 = bt()
    nc.vector.tensor_scalar_mul(out=x5, in0=tab, scalar1=5.0)
    x = bt("x")
    nc.vector.tensor_add(out=x, in0=x5, in1=esq)
    he = bt("he")
    nc.vector.tensor_scalar_mul(out=he, in0=esq, scalar1=0.5)
    lb1, lb2 = ut(), ut()
    nc.vector.tensor_add(out=lb1, in0=x[:, :, 0, :], in1=he[:, :, 1, :])
    nc.vector.tensor_add(out=lb2, in0=x[:, :, 1, :], in1=he[:, :, 0, :])
    resp = ut()
    nc.vector.tensor_tensor(out=resp, in0=iou[:, :, 0, :], in1=iou[:, :, 1, :],
                            op=AL.is_ge)
    dlb = ut()
    nc.vector.tensor_sub(out=dlb, in0=lb1, in1=lb2)
    sd = ut()
    nc.vector.tensor_mul(out=sd, in0=dlb, in1=resp)
    sel = ut("sel")
    nc.vector.tensor_add(out=sel, in0=lb2, in1=sd)

    nq = ut("nq")
    nc.vector.tensor_add(out=nq, in0=sq49[:, :, 0, :], in1=sq49[:, :, 1, :])

    # ---- cls tree ----
    ta = treep.tile([P, 8, 10, K], BF16, tag="ta", name="ta")[:, 0:S]
    nc.vector.tensor_add(out=ta, in0=pcls[:, :, 0:10, :], in1=pcls[:, :, 10:20, :])
    tb = treep.tile([P, 8, 4, K], BF16, tag="tb", name="tb")[:, 0:S]
    nc.vector.tensor_add(out=tb, in0=ta[:, :, 0:4, :], in1=ta[:, :, 4:8, :])
    tc2 = bt("tc2")
    nc.vector.tensor_add(out=tc2, in0=tb[:, :, 0:2, :], in1=tb[:, :, 2:4, :])
    td = ut()
    nc.vector.tensor_add(out=td, in0=tc2[:, :, 0, :], in1=tc2[:, :, 1, :])
    te = ut()
    nc.vector.tensor_add(out=te, in0=ta[:, :, 8, :], in1=ta[:, :, 9, :])
    clsc = ut("clsc")
    nc.vector.tensor_add(out=clsc, in0=td, in1=te)

    # ---- combine + accumulate ----
    nqh = ut()
    nc.vector.tensor_scalar_mul(out=nqh, in0=nq, scalar1=0.5)
    w1 = ut()
    nc.vector.tensor_sub(out=w1, in0=sel, in1=nqh)
    w2 = ut()
    nc.vector.tensor_add(out=w2, in0=clsc, in1=w1)
    wm = ut()
    nc.vector.tensor_mul(out=wm, in0=w2, in1=mask)
    nc.vector.tensor_add(out=ACC[:, 0:S], in0=ACC[:, 0:S], in1=wm)
    nc.vector.tensor_add(out=ACCN[:, 0:S], in0=ACCN[:, 0:S], in1=nq)


_NC_CACHE = None


def _get_nc():
    global _NC_CACHE
    if _NC_CACHE is None:
        _NC_CACHE = _build()
    return _NC_CACHE


def _make_in_maps(pred: np.ndarray, labels: np.ndarray):
    pred = np.ascontiguousarray(pred, dtype=np.float32).reshape(B, 30, K)
    labels = np.ascontiguousarray(labels, dtype=np.float32).reshape(B, 30, K)
    in_maps = []
    for i in range(N_CORES):
        sl = slice(i * BC, (i + 1) * BC)
        in_maps.append({"pred": pred[sl], "labels": labels[sl]})
    return in_maps


def _reduce_results(results) -> np.ndarray:
    total = np.float64(0.0)
    for i in range(N_CORES):
        acc = results[i]["acc"].astype(np.float64)
        total += acc[:, 0].sum() + 0.5 * acc[:, 1].sum()
    return np.asarray(np.float32(total / B))


def kernel(pred: np.ndarray, labels: np.ndarray) -> np.ndarray:
    nc = _get_nc()
    in_maps = _make_in_maps(pred, labels)
    res = run_bass_kernel_spmd(nc, in_maps, core_ids=list(range(N_CORES)),
                               trace=False)
    return _reduce_results(res.results)
